# revision 15
# baseline (speedup 1.0000x reference)
"""Causal attention (QKV proj + softmax(QK^T/sqrt(d))V) on 8 TRN2 NeuronCores.

Sharding: data-parallel over batch (B=8, one batch element per core).
Per-core kernel, all matmuls in float32r (fast fp32 mode; measured
faster per column than bf16 on this part -- bf16 triggers FWL weight
loads that collide with moving-operand streaming):
  warmup: a dozen dummy N=512 matmuls at program start trip the PE HAM
          clock gate to 8/8 before real work arrives; single warm
          matmuls are sprinkled into long transpose stretches so the
          activity monitor never re-throttles.
  phase 1a: x -> x^T via PE transposes, INTERLEAVED with the eb=0 Q
          projections so the PE never sits idle on DMA; Q^T -> DRAM
          scratch (supertile 0 evicted straight to SBUF); K^T -> SBUF
          resident.
  phase 1b: V = x @ Wv evicted IN PLACE over the x^T slab of the same
          t-block (x^T morphs into resident V, no roundtrip).
  phase 2: per 512-wide query supertile: S^T = K Q^T narrowed on the
          diagonal band to the unmasked query range, exp on ACT with
          fused 1/sqrt(D) scale, 128x128 triangular mask on diagonal
          blocks only, P@V with interleaved ones-matmul row sums on PE,
          reciprocal normalize, store (rings rotated).
"""

import numpy as np

T = 2048
D = 1024
E = 1024
N_CORES = 8
P = 128
TS = 512  # t-slice / supertile width
SCALE = 1.0 / 32.0  # 1/sqrt(D)

DC = D // P  # 8 d-chunks
EC = E // P  # 8 e-chunks
TB = T // P  # 16 t-blocks of 128
NTS = T // TS  # 4 t-slices of 512
JB = TS // P  # 4 q-blocks per supertile
QB = TB // 4  # pt part size in k-blocks


def _attention_kernel(ctx, tc, out, x, wq, wk, wv):
    import concourse.bass as bass
    from concourse import mybir
    from concourse.bass import ts
    from concourse.masks import make_identity

    nc = tc.nc
    f32 = mybir.dt.float32
    f32r = mybir.dt.float32r
    AF = mybir.ActivationFunctionType

    # ---- DRAM scratch ----
    dram = ctx.enter_context(tc.tile_pool(name="dram", bufs=1, space="DRAM"))
    qdram = dram.tile([EC, P, T], f32r)  # Q^T[e,t], e = ec*128 + ep

    # ---- left-side SBUF pools ----
    const = ctx.enter_context(tc.tile_pool(name="const", bufs=1))
    ones_f32 = const.tile([P, 2], f32)
    nc.vector.memset(ones_f32[:], 1.0)
    ones_col = const.tile([P, 2], f32r)
    nc.vector.tensor_copy(ones_col[:], ones_f32[:])
    # warm the ACT exp table set at program start (off the critical path)
    exp_warm = const.tile([P, 2], f32)
    nc.scalar.activation(exp_warm[:], ones_f32[:], AF.Exp)
    identity_f32 = const.tile([P, P], f32)
    make_identity(nc, identity_f32[:])
    identity = const.tile([P, P], f32r)
    nc.vector.tensor_copy(identity[:], identity_f32[:])
    identity_bf = const.tile([P, P], mybir.dt.bfloat16)
    nc.vector.tensor_copy(identity_bf[:], identity_f32[:])

    # 128x128 triangular mask (keep where col - p >= 0) for the diagonal
    # 128-col sub-block of each narrowed band S^T block.
    mask_f32 = const.tile([P, P], f32)
    nc.gpsimd.memset(mask_f32[:], 1.0)
    nc.gpsimd.affine_select(
        out=mask_f32[:],
        in_=mask_f32[:],
        compare_op=mybir.AluOpType.is_ge,
        fill=0.0,
        base=0,
        pattern=[[1, P]],
        channel_multiplier=-1,
    )
    mask_diag = const.tile([P, P], f32r)
    nc.vector.tensor_copy(mask_diag[:], mask_f32[:])

    # PE warmup fodder (memset garbage, result never read)
    wu_f32 = const.tile([P, 256], f32)
    nc.gpsimd.memset(wu_f32[:], 0.0)
    wu_rhs = wu_f32[:].bitcast(f32r)
    wu_sink = const.tile([P, 2], f32)

    kt_pool = ctx.enter_context(tc.tile_pool(name="ktres", bufs=1))
    KT = kt_pool.tile([P, EC, T], f32r)  # K^T[e, t], e = ec*128 + ep

    # supertile-0 Q tiles prefetch here (left side, not gated by reuse)
    qt0_pool = ctx.enter_context(tc.tile_pool(name="qt0", bufs=10))

    # ---- right-side work pools ----
    tc.swap_default_side()
    xv_pool = ctx.enter_context(tc.tile_pool(name="xv", bufs=1))
    # x^T t-block-major; after phase 1 each slab is overwritten in place
    # with V[tb] so this same tile is the resident V in phase 2.
    xv = xv_pool.tile([P, TB, DC, P], f32r)  # [dp, tb, dc, tl]
    Vres = xv[:].rearrange("p tb dc e -> p tb (dc e)")  # V[t, e] view
    qstg = tc.alloc_tile_pool(name="qstg", bufs=2)
    xa_pool = tc.alloc_tile_pool(name="xa", bufs=8)
    wqk_pool = tc.alloc_tile_pool(name="wqk", bufs=3)
    wvh_pool = tc.alloc_tile_pool(name="wvh", bufs=2)
    tc.swap_default_side()

    # ---- PSUM pools for phases 0/1 ----
    ps_tp = tc.alloc_tile_pool(name="ps_tp", bufs=3, space="PSUM")
    ps_proj = tc.alloc_tile_pool(name="ps_proj", bufs=5, space="PSUM")

    def warm_mms(n, sink=False):
        """n dummy N=256 matmuls to keep the PE HAM activity window busy."""
        for g in range(n):
            pp = ps_proj.tile([P, TS], f32, tag="pp", name=f"wu{warm_mms.idx}")
            warm_mms.idx += 1
            nc.tensor.matmul(
                pp[:, 0:256], identity[:], wu_rhs, start=True, stop=True
            )
            if sink and g == n - 1:
                nc.vector.tensor_copy(wu_sink[:], pp[:, 0:2])

    warm_mms.idx = 0

    # ===== PE warmup: ~5us of dummy matmuls trips HAM to 8/8 =====
    warm_mms(16, sink=True)

    wq_view0 = wq.bitcast(f32r).rearrange("(dc dp) e -> dp dc e", dp=P)

    # ===== phase 0/1a fused =====
    # x^T is produced with REGULAR bf16 matmuls against a bf16 identity
    # (out = lhsT^T @ I): unlike PE transpose-mode ops these count as PE
    # activity for the HAM clock gate, pipeline in the PE queue, and
    # stream at bf16 rate. x is pre-cast to bf16 per half-row on
    # DVE/ACT; the rounding (~1e-3) is far inside the error budget.
    # 4 transposes land in one [P,512] psum tile and are evicted with a
    # single wide copy (the xv layout keeps dc-consecutive contiguous),
    # cutting the DVE/ACT evict instruction count 4x.
    xvflat = xv[:].rearrange("p tb dc e -> p tb (dc e)")

    def emit_transposes(tsl):
        for tb in range(4 * tsl, 4 * tsl + 4):
            # 4 quarter-row pieces per t-block, issued up front so several
            # DMAs are outstanding (the per-DMA latency is ~5us; depth
            # hides it), then two 4-wide transpose+evict groups.
            pieces = []
            for g in range(4):
                gs = slice(g * 256, (g + 1) * 256)
                xa = xa_pool.tile(
                    [P, 256], f32r, tag="xa", name=f"xa_{tb}_{g}"
                )
                eng = (nc.sync, nc.gpsimd, nc.scalar)[(4 * tb + g) % 3]
                eng.dma_start(xa[:], x[ts(tb, P), gs].bitcast(f32r))
                pieces.append(xa)
            for h in range(2):
                ptw = ps_tp.tile([P, TS], f32r, tag="ptw", name=f"ptw_{tb}_{h}")
                for dd in range(4):
                    nc.tensor.transpose(
                        ptw[:, ts(dd, P)],
                        pieces[2 * h + dd // 2][:, ts(dd % 2, P)],
                        identity[:],
                    )
                if h == 0:
                    nc.vector.tensor_copy(xvflat[:, tb, 0:TS], ptw[:])
                else:
                    nc.scalar.copy(xvflat[:, tb, TS : 2 * TS], ptw[:])
            warm_mms(1)

    qts0 = [
        qt0_pool.tile([P, TS], f32r, tag="qt0", name=f"qts0_{ec}")
        for ec in range(EC)
    ]

    def emit_proj(w_ap, is_q, eb, wr, tsl):
        pp = ps_proj.tile([P, TS], f32)
        for dc in range(DC):
            nc.tensor.matmul(
                pp[:],
                wr[:, dc, :],
                xv[:, 4 * tsl : 4 * tsl + 4, dc, :],
                start=(dc == 0),
                stop=(dc == DC - 1),
            )
        if is_q and tsl == 0:
            nc.vector.tensor_copy(qts0[eb][:], pp[:])
        elif is_q:
            qst = qstg.tile([P, TS], f32r, tag="qstage")
            nc.vector.tensor_copy(qst[:], pp[:])
            nc.scalar.dma_start(qdram[eb, :, ts(tsl, TS)], qst[:])
        elif tsl % 2 == 0:
            nc.vector.tensor_copy(KT[:, eb, ts(tsl, TS)], pp[:])
        else:
            nc.scalar.copy(KT[:, eb, ts(tsl, TS)], pp[:])

    # eb=0 of Q interleaves with the four transpose bursts; its weight
    # slices are loaded after the first x tiles so they don't head-block
    # the DMA rings.
    wk_view0 = wk.bitcast(f32r).rearrange("(dc dp) e -> dp dc e", dp=P)
    wr_q01 = []
    wr_k0 = None
    for tsl in range(NTS):
        emit_transposes(tsl)
        if tsl == 0:
            for eb in range(2):
                wr = wqk_pool.tile(
                    [P, DC, P], f32r, tag="wqk", name=f"wrq_{eb}"
                )
                eng = nc.scalar if eb == 0 else nc.sync
                eng.dma_start(wr[:], wq_view0[:, :, ts(eb, P)])
                wr_q01.append(wr)
            wr_k0 = wqk_pool.tile([P, DC, P], f32r, tag="wqk", name="wrk_0")
            nc.gpsimd.dma_start(wr_k0[:], wk_view0[:, :, ts(0, P)])
        emit_proj(wq, True, 0, wr_q01[0], tsl)
        emit_proj(wk, False, 0, wr_k0, tsl)
    # remaining Q (eb 1..7), then all K
    for w_ap, is_q in ((wq, True), (wk, False)):
        w_view = w_ap.bitcast(f32r).rearrange("(dc dp) e -> dp dc e", dp=P)
        for eb in range(EC):
            if eb == 0:
                continue
            if is_q and eb == 1:
                wr = wr_q01[1]
            else:
                wr = wqk_pool.tile([P, DC, P], f32r, tag="wqk")
                weng = (nc.sync, nc.gpsimd, nc.scalar)[eb % 3]
                weng.dma_start(wr[:], w_view[:, :, ts(eb, P)])
            for tsl in range(NTS):
                emit_proj(w_ap, is_q, eb, wr, tsl)

    # ========== phase 1b: V = x @ Wv, evicted in place over x^T ==========
    # tb-outer with both Wv halves resident: both psums must be computed
    # before the in-place evicts may overwrite this t-block's x^T slab.
    wv_view = wv.bitcast(f32r).rearrange("(dc dp) e -> dp dc e", dp=P)
    wvhs = []
    for eh in range(E // TS):
        wvh = wvh_pool.tile([P, DC, TS], f32r, tag="wvh", name=f"wvh_{eh}")
        nc.sync.dma_start(wvh[:], wv_view[:, :, ts(eh, TS)])
        wvhs.append(wvh)
    for tb in range(TB):
        pps = []
        for eh in range(E // TS):
            pp = ps_proj.tile([P, TS], f32)
            for dc in range(DC):
                nc.tensor.matmul(
                    pp[:],
                    xv[:, tb, dc, :],
                    wvhs[eh][:, dc, :],
                    start=(dc == 0),
                    stop=(dc == DC - 1),
                )
            pps.append(pp)
        # in-place evicts over the x^T slab of this t-block (WAR: both
        # psum groups above have read the slab before these run)
        nc.scalar.copy(Vres[:, tb, ts(0, TS)], pps[0][:])
        nc.vector.tensor_copy(Vres[:, tb, ts(1, TS)], pps[1][:])

    wvh_pool.release()
    wqk_pool.release()
    xa_pool.release()
    qstg.release()
    ps_proj.release()
    ps_tp.release()

    # ================= phase 2: attention =================
    ps_s = tc.alloc_tile_pool(name="ps_s", bufs=4, space="PSUM")
    ps_o = tc.alloc_tile_pool(name="ps_o", bufs=2, space="PSUM")
    ps_sum = tc.alloc_tile_pool(name="ps_sum", bufs=2, space="PSUM")

    tc.swap_default_side()
    pt_pool = ctx.enter_context(tc.tile_pool(name="pt", bufs=5))
    rs_pool = ctx.enter_context(tc.tile_pool(name="rs", bufs=8))
    ostg = ctx.enter_context(tc.tile_pool(name="ostg", bufs=3))
    tc.swap_default_side()

    store_rings = (nc.scalar, nc.gpsimd, nc.sync)
    nstores = 0

    qts_next = qts0
    for sup in range(NTS):
        nkb = JB * sup + JB  # key blocks 0..nkb-1
        qts = qts_next
        pt_parts = [
            pt_pool.tile([P, QB, TS], f32r, tag="pt", name=f"ptp_{sup}_0")
        ]

        # --- S^T blocks + exp + causal mask (band blocks narrowed) ---
        for k in range(nkb):
            j = k - JB * sup  # >= 0 on the diagonal band
            lo = max(j, 0) * P  # first unmasked query col in supertile
            ssp = ps_s.tile([P, TS], f32)
            for ec in range(EC):
                nc.tensor.matmul(
                    ssp[:, lo:TS],
                    KT[:, ec, ts(k, P)],
                    qts[ec][:, lo:TS],
                    start=(ec == 0),
                    stop=(ec == EC - 1),
                )
            if k // QB >= len(pt_parts):
                pt_parts.append(
                    pt_pool.tile(
                        [P, QB, TS], f32r, tag="pt",
                        name=f"ptp_{sup}_{k // QB}",
                    )
                )
            pk = pt_parts[k // QB][:, k % QB, :]
            nc.scalar.activation(pk[:, lo:TS], ssp[:, lo:TS], AF.Exp, scale=SCALE)
            if j >= 0:
                nc.vector.tensor_mul(
                    pk[:, lo : lo + P], pk[:, lo : lo + P], mask_diag[:]
                )

        # Prefetch the NEXT supertile's Q tiles now: their ring slots sit
        # ahead of this supertile's output stores, and the pool slots they
        # rotate into were released when this S phase finished reading.
        if sup + 1 < NTS:
            qts_next = []
            for ec in range(EC):
                q1 = qt0_pool.tile(
                    [P, TS], f32r, tag="qt0", name=f"qt_{sup + 1}_{ec}"
                )
                eng = nc.sync if ec % 2 == 0 else nc.gpsimd
                eng.dma_start(q1[:], qdram[ec, :, ts(sup + 1, TS)])
                qts_next.append(q1)

        # --- P @ V (+ row sums interleaved in eh=0), normalize, store ---
        rss = {}
        for eh in range(E // TS):
            for jq in range(JB):
                qb = JB * sup + jq
                nk = qb + 1
                po = ps_o.tile([P, TS], f32)
                if eh == 0:
                    pos = ps_sum.tile([P, 2], f32)
                for k in range(nk):
                    lhsT = pt_parts[k // QB][:, k % QB, ts(jq, P)]
                    nc.tensor.matmul(
                        po[:],
                        lhsT,
                        Vres[:, k, ts(eh, TS)],
                        start=(k == 0),
                        stop=(k == nk - 1),
                    )
                    if eh == 0:
                        nc.tensor.matmul(
                            pos[:],
                            lhsT,
                            ones_col[:],
                            start=(k == 0),
                            stop=(k == nk - 1),
                        )
                if eh == 0:
                    rs = rs_pool.tile(
                        [P, 1], f32, tag="rs", name=f"rs_{sup}_{jq}"
                    )
                    nc.vector.reciprocal(rs[:], pos[:, 0:1])
                    rss[jq] = rs
                ost = ostg.tile([P, TS], f32, tag="ostage")
                if eh == 0:
                    nc.scalar.activation(
                        ost[:], po[:], AF.Copy, scale=rss[jq][:]
                    )
                else:
                    nc.vector.tensor_scalar_mul(ost[:], po[:], rss[jq][:])
                store_rings[nstores % 3].dma_start(
                    out[ts(qb, P), ts(eh, TS)], ost[:]
                )
                nstores += 1

    ps_sum.release()
    ps_o.release()
    ps_s.release()


def build_program():
    from contextlib import ExitStack

    import concourse.bacc as bacc
    import concourse.tile as tile
    from concourse import mybir

    nc = bacc.Bacc("TRN2", target_bir_lowering=False, debug=False)
    f32 = mybir.dt.float32
    x = nc.dram_tensor("x", [T, D], f32, kind="ExternalInput").ap()
    wq = nc.dram_tensor("Wq", [D, E], f32, kind="ExternalInput").ap()
    wk = nc.dram_tensor("Wk", [D, E], f32, kind="ExternalInput").ap()
    wv = nc.dram_tensor("Wv", [D, E], f32, kind="ExternalInput").ap()
    out = nc.dram_tensor("out", [T, E], f32, kind="ExternalOutput").ap()

    with tile.TileContext(nc) as tc:
        with ExitStack() as ctx:
            _attention_kernel(ctx, tc, out, x, wq, wk, wv)
    nc.compile()
    return nc


def kernel(x, Wq, Wk, Wv, _trace=False):
    from concourse.bass_utils import run_bass_kernel_spmd

    x = np.ascontiguousarray(np.asarray(x), dtype=np.float32)
    Wq = np.ascontiguousarray(np.asarray(Wq), dtype=np.float32)
    Wk = np.ascontiguousarray(np.asarray(Wk), dtype=np.float32)
    Wv = np.ascontiguousarray(np.asarray(Wv), dtype=np.float32)
    assert x.shape == (N_CORES, T, D), x.shape

    nc = build_program()
    in_maps = [
        {"x": np.ascontiguousarray(x[b]), "Wq": Wq, "Wk": Wk, "Wv": Wv}
        for b in range(N_CORES)
    ]
    last_err = None
    for attempt in range(3):
        try:
            res = run_bass_kernel_spmd(
                nc, in_maps, core_ids=list(range(N_CORES)), trace=_trace
            )
            break
        except Exception as e:  # transient device wedge: retry
            last_err = e
            import time

            time.sleep(5.0 * (attempt + 1))
    else:
        raise last_err
    out = np.stack([res.results[b]["out"] for b in range(N_CORES)], axis=0)
    if _trace:
        kernel.last_results = res
    return out


kernel.last_results = None


# revision 17
# speedup vs baseline: 1.0369x; 1.0369x over previous
"""Causal attention (QKV proj + softmax(QK^T/sqrt(d))V) on 8 TRN2 NeuronCores.

Sharding: data-parallel over batch (B=8, one batch element per core).
Per-core kernel, all matmuls in float32r (fast fp32 mode; measured
faster per column than bf16 on this part -- bf16 triggers FWL weight
loads that collide with moving-operand streaming):
  warmup: a dozen dummy N=512 matmuls at program start trip the PE HAM
          clock gate to 8/8 before real work arrives; single warm
          matmuls are sprinkled into long transpose stretches so the
          activity monitor never re-throttles.
  phase 1a: x -> x^T via PE transposes, INTERLEAVED with the eb=0 Q
          projections so the PE never sits idle on DMA; Q^T -> DRAM
          scratch (supertile 0 evicted straight to SBUF); K^T -> SBUF
          resident.
  phase 1b: V = x @ Wv evicted IN PLACE over the x^T slab of the same
          t-block (x^T morphs into resident V, no roundtrip).
  phase 2: per 512-wide query supertile: S^T = K Q^T narrowed on the
          diagonal band to the unmasked query range, exp on ACT with
          fused 1/sqrt(D) scale, 128x128 triangular mask on diagonal
          blocks only, P@V with interleaved ones-matmul row sums on PE,
          reciprocal normalize, store (rings rotated).
"""

import numpy as np

T = 2048
D = 1024
E = 1024
N_CORES = 8
P = 128
TS = 512  # t-slice / supertile width
SCALE = 1.0 / 32.0  # 1/sqrt(D)

DC = D // P  # 8 d-chunks
EC = E // P  # 8 e-chunks
TB = T // P  # 16 t-blocks of 128
NTS = T // TS  # 4 t-slices of 512
JB = TS // P  # 4 q-blocks per supertile
QB = TB // 4  # pt part size in k-blocks


def _attention_kernel(ctx, tc, out, x, wq, wk, wv):
    import concourse.bass as bass
    from concourse import mybir
    from concourse.bass import ts
    from concourse.masks import make_identity

    nc = tc.nc
    f32 = mybir.dt.float32
    f32r = mybir.dt.float32r
    AF = mybir.ActivationFunctionType

    # ---- DRAM scratch ----
    dram = ctx.enter_context(tc.tile_pool(name="dram", bufs=1, space="DRAM"))
    qdram = dram.tile([EC, P, T], f32r)  # Q^T[e,t], e = ec*128 + ep

    # ---- left-side SBUF pools ----
    const = ctx.enter_context(tc.tile_pool(name="const", bufs=1))
    ones_f32 = const.tile([P, 2], f32)
    nc.vector.memset(ones_f32[:], 1.0)
    ones_col = const.tile([P, 2], f32r)
    nc.vector.tensor_copy(ones_col[:], ones_f32[:])
    # warm the ACT exp table set at program start (off the critical path)
    exp_warm = const.tile([P, 2], f32)
    nc.scalar.activation(exp_warm[:], ones_f32[:], AF.Exp)
    identity_f32 = const.tile([P, P], f32)
    make_identity(nc, identity_f32[:])
    identity = const.tile([P, P], f32r)
    nc.vector.tensor_copy(identity[:], identity_f32[:])
    identity_bf = const.tile([P, P], mybir.dt.bfloat16)
    nc.vector.tensor_copy(identity_bf[:], identity_f32[:])

    # 128x128 triangular mask (keep where col - p >= 0) for the diagonal
    # 128-col sub-block of each narrowed band S^T block.
    mask_f32 = const.tile([P, P], f32)
    nc.gpsimd.memset(mask_f32[:], 1.0)
    nc.gpsimd.affine_select(
        out=mask_f32[:],
        in_=mask_f32[:],
        compare_op=mybir.AluOpType.is_ge,
        fill=0.0,
        base=0,
        pattern=[[1, P]],
        channel_multiplier=-1,
    )
    mask_diag = const.tile([P, P], f32r)
    nc.vector.tensor_copy(mask_diag[:], mask_f32[:])

    # PE warmup fodder (memset garbage, result never read)
    wu_f32 = const.tile([P, 256], f32)
    nc.gpsimd.memset(wu_f32[:], 0.0)
    wu_rhs = wu_f32[:].bitcast(f32r)
    wu_sink = const.tile([P, 2], f32)

    kt_pool = ctx.enter_context(tc.tile_pool(name="ktres", bufs=1))
    KT = kt_pool.tile([P, EC, T], f32r)  # K^T[e, t], e = ec*128 + ep

    # supertile-0 Q tiles prefetch here (left side, not gated by reuse)
    qt0_pool = ctx.enter_context(tc.tile_pool(name="qt0", bufs=10))

    # ---- right-side work pools ----
    tc.swap_default_side()
    xv_pool = ctx.enter_context(tc.tile_pool(name="xv", bufs=1))
    # x^T t-block-major; after phase 1 each slab is overwritten in place
    # with V[tb] so this same tile is the resident V in phase 2.
    xv = xv_pool.tile([P, TB, DC, P], f32r)  # [dp, tb, dc, tl]
    Vres = xv[:].rearrange("p tb dc e -> p tb (dc e)")  # V[t, e] view
    qstg = tc.alloc_tile_pool(name="qstg", bufs=2)
    xa_pool = tc.alloc_tile_pool(name="xa", bufs=8)
    wqk_pool = tc.alloc_tile_pool(name="wqk", bufs=3)
    wvh_pool = tc.alloc_tile_pool(name="wvh", bufs=2)
    tc.swap_default_side()

    # ---- PSUM pools for phases 0/1 ----
    ps_tp = tc.alloc_tile_pool(name="ps_tp", bufs=3, space="PSUM")
    ps_proj = tc.alloc_tile_pool(name="ps_proj", bufs=5, space="PSUM")

    def warm_mms(n, sink=False):
        """n dummy N=256 matmuls to keep the PE HAM activity window busy."""
        for g in range(n):
            pp = ps_proj.tile([P, TS], f32, tag="pp", name=f"wu{warm_mms.idx}")
            warm_mms.idx += 1
            nc.tensor.matmul(
                pp[:, 0:256], identity[:], wu_rhs, start=True, stop=True
            )
            if sink and g == n - 1:
                nc.vector.tensor_copy(wu_sink[:], pp[:, 0:2])

    warm_mms.idx = 0

    # ===== PE warmup: ~5us of dummy matmuls trips HAM to 8/8 =====
    warm_mms(16, sink=True)

    wq_view0 = wq.bitcast(f32r).rearrange("(dc dp) e -> dp dc e", dp=P)

    # ===== phase 0/1a fused =====
    # x^T is produced with REGULAR bf16 matmuls against a bf16 identity
    # (out = lhsT^T @ I): unlike PE transpose-mode ops these count as PE
    # activity for the HAM clock gate, pipeline in the PE queue, and
    # stream at bf16 rate. x is pre-cast to bf16 per half-row on
    # DVE/ACT; the rounding (~1e-3) is far inside the error budget.
    # 4 transposes land in one [P,512] psum tile and are evicted with a
    # single wide copy (the xv layout keeps dc-consecutive contiguous),
    # cutting the DVE/ACT evict instruction count 4x.
    xvflat = xv[:].rearrange("p tb dc e -> p tb (dc e)")

    def emit_transposes(tsl):
        for tb in range(4 * tsl, 4 * tsl + 4):
            # 4 quarter-row pieces per t-block, issued up front so several
            # DMAs are outstanding (the per-DMA latency is ~5us; depth
            # hides it), then two 4-wide transpose+evict groups.
            pieces = []
            for g in range(4):
                gs = slice(g * 256, (g + 1) * 256)
                xa = xa_pool.tile(
                    [P, 256], f32r, tag="xa", name=f"xa_{tb}_{g}"
                )
                eng = (nc.sync, nc.gpsimd, nc.scalar)[(4 * tb + g) % 3]
                eng.dma_start(xa[:], x[ts(tb, P), gs].bitcast(f32r))
                pieces.append(xa)
            for h in range(2):
                ptw = ps_tp.tile([P, TS], f32r, tag="ptw", name=f"ptw_{tb}_{h}")
                for dd in range(4):
                    nc.tensor.transpose(
                        ptw[:, ts(dd, P)],
                        pieces[2 * h + dd // 2][:, ts(dd % 2, P)],
                        identity[:],
                    )
                if h == 0:
                    nc.vector.tensor_copy(xvflat[:, tb, 0:TS], ptw[:])
                else:
                    nc.scalar.copy(xvflat[:, tb, TS : 2 * TS], ptw[:])
            warm_mms(1)

    qts0 = [
        qt0_pool.tile([P, TS], f32r, tag="qt0", name=f"qts0_{ec}")
        for ec in range(EC)
    ]

    def emit_proj(w_ap, is_q, eb, wr, tsl):
        pp = ps_proj.tile([P, TS], f32)
        for dc in range(DC):
            nc.tensor.matmul(
                pp[:],
                wr[:, dc, :],
                xv[:, 4 * tsl : 4 * tsl + 4, dc, :],
                start=(dc == 0),
                stop=(dc == DC - 1),
            )
        if is_q and tsl == 0:
            nc.vector.tensor_copy(qts0[eb][:], pp[:])
        elif is_q:
            qst = qstg.tile([P, TS], f32r, tag="qstage")
            nc.vector.tensor_copy(qst[:], pp[:])
            nc.scalar.dma_start(qdram[eb, :, ts(tsl, TS)], qst[:])
        elif tsl % 2 == 0:
            nc.vector.tensor_copy(KT[:, eb, ts(tsl, TS)], pp[:])
        else:
            nc.scalar.copy(KT[:, eb, ts(tsl, TS)], pp[:])

    # eb=0 of Q interleaves with the four transpose bursts; its weight
    # slices are loaded after the first x tiles so they don't head-block
    # the DMA rings.
    wk_view0 = wk.bitcast(f32r).rearrange("(dc dp) e -> dp dc e", dp=P)
    wr_q01 = []
    wr_k0 = None
    for tsl in range(NTS):
        emit_transposes(tsl)
        if tsl == 0:
            for eb in range(2):
                wr = wqk_pool.tile(
                    [P, DC, P], f32r, tag="wqk", name=f"wrq_{eb}"
                )
                eng = nc.scalar if eb == 0 else nc.sync
                eng.dma_start(wr[:], wq_view0[:, :, ts(eb, P)])
                wr_q01.append(wr)
            wr_k0 = wqk_pool.tile([P, DC, P], f32r, tag="wqk", name="wrk_0")
            nc.gpsimd.dma_start(wr_k0[:], wk_view0[:, :, ts(0, P)])
        emit_proj(wq, True, 0, wr_q01[0], tsl)
        emit_proj(wk, False, 0, wr_k0, tsl)
    # remaining Q (eb 1..7), then all K
    for w_ap, is_q in ((wq, True), (wk, False)):
        w_view = w_ap.bitcast(f32r).rearrange("(dc dp) e -> dp dc e", dp=P)
        for eb in range(EC):
            if eb == 0:
                continue
            if is_q and eb == 1:
                wr = wr_q01[1]
            else:
                wr = wqk_pool.tile([P, DC, P], f32r, tag="wqk")
                weng = (nc.sync, nc.gpsimd, nc.scalar)[eb % 3]
                weng.dma_start(wr[:], w_view[:, :, ts(eb, P)])
            for tsl in range(NTS):
                emit_proj(w_ap, is_q, eb, wr, tsl)

    # ========== phase 1b: V = x @ Wv, evicted in place over x^T ==========
    # tb-outer with both Wv halves resident: both psums must be computed
    # before the in-place evicts may overwrite this t-block's x^T slab.
    wv_view = wv.bitcast(f32r).rearrange("(dc dp) e -> dp dc e", dp=P)
    wvhs = []
    for eh in range(E // TS):
        wvh = wvh_pool.tile([P, DC, TS], f32r, tag="wvh", name=f"wvh_{eh}")
        nc.sync.dma_start(wvh[:], wv_view[:, :, ts(eh, TS)])
        wvhs.append(wvh)
    for tb in range(TB):
        pps = []
        for eh in range(E // TS):
            pp = ps_proj.tile([P, TS], f32)
            for dc in range(DC):
                nc.tensor.matmul(
                    pp[:],
                    xv[:, tb, dc, :],
                    wvhs[eh][:, dc, :],
                    start=(dc == 0),
                    stop=(dc == DC - 1),
                )
            pps.append(pp)
        # in-place evicts over the x^T slab of this t-block (WAR: both
        # psum groups above have read the slab before these run)
        nc.scalar.copy(Vres[:, tb, ts(0, TS)], pps[0][:])
        nc.vector.tensor_copy(Vres[:, tb, ts(1, TS)], pps[1][:])

    wvh_pool.release()
    wqk_pool.release()
    xa_pool.release()
    qstg.release()
    ps_proj.release()
    ps_tp.release()

    # ================= phase 2: attention =================
    ps_s = tc.alloc_tile_pool(name="ps_s", bufs=4, space="PSUM")
    ps_o = tc.alloc_tile_pool(name="ps_o", bufs=2, space="PSUM")
    ps_sum = tc.alloc_tile_pool(name="ps_sum", bufs=2, space="PSUM")

    tc.swap_default_side()
    pt_pool = ctx.enter_context(tc.tile_pool(name="pt", bufs=5))
    rs_pool = ctx.enter_context(tc.tile_pool(name="rs", bufs=8))
    tc.swap_default_side()
    # ostg on the LEFT side: during PV the PE streams V and probs from the
    # right side, so store-DMA reads of ost tiles don't contend.
    ostg = ctx.enter_context(tc.tile_pool(name="ostg", bufs=3))

    store_rings = (nc.scalar, nc.gpsimd, nc.sync)
    nstores = 0

    for sup in range(NTS):
        nkb = JB * sup + JB  # key blocks 0..nkb-1
        if sup == 0:
            qts = qts0
        else:
            qts = []
            for ec in range(EC):
                q1 = qt0_pool.tile([P, TS], f32r, tag="qt0", name=f"qt_{sup}_{ec}")
                eng = nc.sync if ec % 2 == 0 else nc.gpsimd
                eng.dma_start(q1[:], qdram[ec, :, ts(sup, TS)])
                qts.append(q1)
        pt_parts = [
            pt_pool.tile([P, QB, TS], f32r, tag="pt", name=f"ptp_{sup}_0")
        ]

        # --- S^T blocks + exp + causal mask (band blocks narrowed) ---
        for k in range(nkb):
            j = k - JB * sup  # >= 0 on the diagonal band
            lo = max(j, 0) * P  # first unmasked query col in supertile
            ssp = ps_s.tile([P, TS], f32)
            for ec in range(EC):
                nc.tensor.matmul(
                    ssp[:, lo:TS],
                    KT[:, ec, ts(k, P)],
                    qts[ec][:, lo:TS],
                    start=(ec == 0),
                    stop=(ec == EC - 1),
                )
            if k // QB >= len(pt_parts):
                pt_parts.append(
                    pt_pool.tile(
                        [P, QB, TS], f32r, tag="pt",
                        name=f"ptp_{sup}_{k // QB}",
                    )
                )
            pk = pt_parts[k // QB][:, k % QB, :]
            nc.scalar.activation(pk[:, lo:TS], ssp[:, lo:TS], AF.Exp, scale=SCALE)
            if j >= 0:
                nc.vector.tensor_mul(
                    pk[:, lo : lo + P], pk[:, lo : lo + P], mask_diag[:]
                )

        # --- P @ V (+ row sums interleaved in eh=0), normalize, store ---
        rss = {}
        for eh in range(E // TS):
            for jq in range(JB):
                qb = JB * sup + jq
                nk = qb + 1
                po = ps_o.tile([P, TS], f32)
                if eh == 0:
                    pos = ps_sum.tile([P, 2], f32)
                for k in range(nk):
                    lhsT = pt_parts[k // QB][:, k % QB, ts(jq, P)]
                    nc.tensor.matmul(
                        po[:],
                        lhsT,
                        Vres[:, k, ts(eh, TS)],
                        start=(k == 0),
                        stop=(k == nk - 1),
                    )
                    if eh == 0:
                        nc.tensor.matmul(
                            pos[:],
                            lhsT,
                            ones_col[:],
                            start=(k == 0),
                            stop=(k == nk - 1),
                        )
                if eh == 0:
                    rs = rs_pool.tile(
                        [P, 1], f32, tag="rs", name=f"rs_{sup}_{jq}"
                    )
                    nc.vector.reciprocal(rs[:], pos[:, 0:1])
                    rss[jq] = rs
                ost = ostg.tile([P, TS], f32, tag="ostage")
                if eh == 0:
                    nc.scalar.activation(
                        ost[:], po[:], AF.Copy, scale=rss[jq][:]
                    )
                else:
                    nc.vector.tensor_scalar_mul(ost[:], po[:], rss[jq][:])
                store_rings[nstores % 3].dma_start(
                    out[ts(qb, P), ts(eh, TS)], ost[:]
                )
                nstores += 1

    ps_sum.release()
    ps_o.release()
    ps_s.release()


def build_program():
    from contextlib import ExitStack

    import concourse.bacc as bacc
    import concourse.tile as tile
    from concourse import mybir

    nc = bacc.Bacc("TRN2", target_bir_lowering=False, debug=False)
    f32 = mybir.dt.float32
    x = nc.dram_tensor("x", [T, D], f32, kind="ExternalInput").ap()
    wq = nc.dram_tensor("Wq", [D, E], f32, kind="ExternalInput").ap()
    wk = nc.dram_tensor("Wk", [D, E], f32, kind="ExternalInput").ap()
    wv = nc.dram_tensor("Wv", [D, E], f32, kind="ExternalInput").ap()
    out = nc.dram_tensor("out", [T, E], f32, kind="ExternalOutput").ap()

    with tile.TileContext(nc) as tc:
        with ExitStack() as ctx:
            _attention_kernel(ctx, tc, out, x, wq, wk, wv)
    nc.compile()
    return nc


def kernel(x, Wq, Wk, Wv, _trace=False):
    from concourse.bass_utils import run_bass_kernel_spmd

    x = np.ascontiguousarray(np.asarray(x), dtype=np.float32)
    Wq = np.ascontiguousarray(np.asarray(Wq), dtype=np.float32)
    Wk = np.ascontiguousarray(np.asarray(Wk), dtype=np.float32)
    Wv = np.ascontiguousarray(np.asarray(Wv), dtype=np.float32)
    assert x.shape == (N_CORES, T, D), x.shape

    nc = build_program()
    in_maps = [
        {"x": np.ascontiguousarray(x[b]), "Wq": Wq, "Wk": Wk, "Wv": Wv}
        for b in range(N_CORES)
    ]
    last_err = None
    for attempt in range(3):
        try:
            res = run_bass_kernel_spmd(
                nc, in_maps, core_ids=list(range(N_CORES)), trace=_trace
            )
            break
        except Exception as e:  # transient device wedge: retry
            last_err = e
            import time

            time.sleep(5.0 * (attempt + 1))
    else:
        raise last_err
    out = np.stack([res.results[b]["out"] for b in range(N_CORES)], axis=0)
    if _trace:
        kernel.last_results = res
    return out


kernel.last_results = None


# revision 18
# speedup vs baseline: 1.0398x; 1.0028x over previous
"""Causal attention (QKV proj + softmax(QK^T/sqrt(d))V) on 8 TRN2 NeuronCores.

Sharding: data-parallel over batch (B=8, one batch element per core).
Per-core kernel, all matmuls in float32r (fast fp32 mode; measured
faster per column than bf16 on this part -- bf16 triggers FWL weight
loads that collide with moving-operand streaming):
  warmup: a dozen dummy N=512 matmuls at program start trip the PE HAM
          clock gate to 8/8 before real work arrives; single warm
          matmuls are sprinkled into long transpose stretches so the
          activity monitor never re-throttles.
  phase 1a: x -> x^T via PE transposes, INTERLEAVED with the eb=0 Q
          projections so the PE never sits idle on DMA; Q^T -> DRAM
          scratch (supertile 0 evicted straight to SBUF); K^T -> SBUF
          resident.
  phase 1b: V = x @ Wv evicted IN PLACE over the x^T slab of the same
          t-block (x^T morphs into resident V, no roundtrip).
  phase 2: per 512-wide query supertile: S^T = K Q^T narrowed on the
          diagonal band to the unmasked query range, exp on ACT with
          fused 1/sqrt(D) scale, 128x128 triangular mask on diagonal
          blocks only, P@V with interleaved ones-matmul row sums on PE,
          reciprocal normalize, store (rings rotated).
"""

import numpy as np

T = 2048
D = 1024
E = 1024
N_CORES = 8
P = 128
TS = 512  # t-slice / supertile width
SCALE = 1.0 / 32.0  # 1/sqrt(D)

DC = D // P  # 8 d-chunks
EC = E // P  # 8 e-chunks
TB = T // P  # 16 t-blocks of 128
NTS = T // TS  # 4 t-slices of 512
JB = TS // P  # 4 q-blocks per supertile
QB = TB // 4  # pt part size in k-blocks


def _attention_kernel(ctx, tc, out, x, wq, wk, wv):
    import concourse.bass as bass
    from concourse import mybir
    from concourse.bass import ts
    from concourse.masks import make_identity

    nc = tc.nc
    f32 = mybir.dt.float32
    f32r = mybir.dt.float32r
    AF = mybir.ActivationFunctionType

    # ---- DRAM scratch ----
    dram = ctx.enter_context(tc.tile_pool(name="dram", bufs=1, space="DRAM"))
    qdram = dram.tile([EC, P, T], f32r)  # Q^T[e,t], e = ec*128 + ep

    # ---- left-side SBUF pools ----
    const = ctx.enter_context(tc.tile_pool(name="const", bufs=1))
    ones_f32 = const.tile([P, 2], f32)
    nc.vector.memset(ones_f32[:], 1.0)
    ones_col = const.tile([P, 2], f32r)
    nc.vector.tensor_copy(ones_col[:], ones_f32[:])
    # warm the ACT exp table set at program start (off the critical path)
    exp_warm = const.tile([P, 2], f32)
    nc.scalar.activation(exp_warm[:], ones_f32[:], AF.Exp)
    identity_f32 = const.tile([P, P], f32)
    make_identity(nc, identity_f32[:])
    identity = const.tile([P, P], f32r)
    nc.vector.tensor_copy(identity[:], identity_f32[:])
    identity_bf = const.tile([P, P], mybir.dt.bfloat16)
    nc.vector.tensor_copy(identity_bf[:], identity_f32[:])

    # 128x128 triangular mask (keep where col - p >= 0) for the diagonal
    # 128-col sub-block of each narrowed band S^T block.
    mask_f32 = const.tile([P, P], f32)
    nc.gpsimd.memset(mask_f32[:], 1.0)
    nc.gpsimd.affine_select(
        out=mask_f32[:],
        in_=mask_f32[:],
        compare_op=mybir.AluOpType.is_ge,
        fill=0.0,
        base=0,
        pattern=[[1, P]],
        channel_multiplier=-1,
    )
    mask_diag = const.tile([P, P], f32r)
    nc.vector.tensor_copy(mask_diag[:], mask_f32[:])

    # PE warmup fodder (memset garbage, result never read)
    wu_f32 = const.tile([P, 256], f32)
    nc.gpsimd.memset(wu_f32[:], 0.0)
    wu_rhs = wu_f32[:].bitcast(f32r)
    wu_sink = const.tile([P, 2], f32)

    kt_pool = ctx.enter_context(tc.tile_pool(name="ktres", bufs=1))
    KT = kt_pool.tile([P, EC, T], f32r)  # K^T[e, t], e = ec*128 + ep

    # supertile-0 Q tiles prefetch here (left side, not gated by reuse)
    qt0_pool = ctx.enter_context(tc.tile_pool(name="qt0", bufs=10))

    # ---- right-side work pools ----
    tc.swap_default_side()
    xv_pool = ctx.enter_context(tc.tile_pool(name="xv", bufs=1))
    # x^T t-block-major; after phase 1 each slab is overwritten in place
    # with V[tb] so this same tile is the resident V in phase 2.
    xv = xv_pool.tile([P, TB, DC, P], f32r)  # [dp, tb, dc, tl]
    Vres = xv[:].rearrange("p tb dc e -> p tb (dc e)")  # V[t, e] view
    qstg = tc.alloc_tile_pool(name="qstg", bufs=2)
    xa_pool = tc.alloc_tile_pool(name="xa", bufs=8)
    wqk_pool = tc.alloc_tile_pool(name="wqk", bufs=3)
    wvh_pool = tc.alloc_tile_pool(name="wvh", bufs=2)
    tc.swap_default_side()

    # ---- PSUM pools for phases 0/1 ----
    ps_tp = tc.alloc_tile_pool(name="ps_tp", bufs=3, space="PSUM")
    ps_proj = tc.alloc_tile_pool(name="ps_proj", bufs=5, space="PSUM")

    def warm_mms(n, sink=False):
        """n dummy N=256 matmuls to keep the PE HAM activity window busy."""
        for g in range(n):
            pp = ps_proj.tile([P, TS], f32, tag="pp", name=f"wu{warm_mms.idx}")
            warm_mms.idx += 1
            nc.tensor.matmul(
                pp[:, 0:256], identity[:], wu_rhs, start=True, stop=True
            )
            if sink and g == n - 1:
                nc.vector.tensor_copy(wu_sink[:], pp[:, 0:2])

    warm_mms.idx = 0

    # ===== PE warmup: ~5us of dummy matmuls trips HAM to 8/8 =====
    warm_mms(16, sink=True)

    wq_view0 = wq.bitcast(f32r).rearrange("(dc dp) e -> dp dc e", dp=P)

    # ===== phase 0/1a fused =====
    # x^T is produced with REGULAR bf16 matmuls against a bf16 identity
    # (out = lhsT^T @ I): unlike PE transpose-mode ops these count as PE
    # activity for the HAM clock gate, pipeline in the PE queue, and
    # stream at bf16 rate. x is pre-cast to bf16 per half-row on
    # DVE/ACT; the rounding (~1e-3) is far inside the error budget.
    # 4 transposes land in one [P,512] psum tile and are evicted with a
    # single wide copy (the xv layout keeps dc-consecutive contiguous),
    # cutting the DVE/ACT evict instruction count 4x.
    xvflat = xv[:].rearrange("p tb dc e -> p tb (dc e)")

    def emit_transposes(tsl):
        for tb in range(4 * tsl, 4 * tsl + 4):
            # 4 quarter-row pieces per t-block, issued up front so several
            # DMAs are outstanding (the per-DMA latency is ~5us; depth
            # hides it), then two 4-wide transpose+evict groups.
            pieces = []
            for g in range(4):
                gs = slice(g * 256, (g + 1) * 256)
                xa = xa_pool.tile(
                    [P, 256], f32r, tag="xa", name=f"xa_{tb}_{g}"
                )
                eng = (nc.sync, nc.gpsimd)[(4 * tb + g) % 2]
                eng.dma_start(xa[:], x[ts(tb, P), gs].bitcast(f32r))
                pieces.append(xa)
            for h in range(2):
                ptw = ps_tp.tile([P, TS], f32r, tag="ptw", name=f"ptw_{tb}_{h}")
                for dd in range(4):
                    nc.tensor.transpose(
                        ptw[:, ts(dd, P)],
                        pieces[2 * h + dd // 2][:, ts(dd % 2, P)],
                        identity[:],
                    )
                if h == 0:
                    nc.vector.tensor_copy(xvflat[:, tb, 0:TS], ptw[:])
                else:
                    nc.scalar.copy(xvflat[:, tb, TS : 2 * TS], ptw[:])
            warm_mms(1)

    qts0 = [
        qt0_pool.tile([P, TS], f32r, tag="qt0", name=f"qts0_{ec}")
        for ec in range(EC)
    ]

    def emit_proj(w_ap, is_q, eb, wr, tsl):
        pp = ps_proj.tile([P, TS], f32)
        for dc in range(DC):
            nc.tensor.matmul(
                pp[:],
                wr[:, dc, :],
                xv[:, 4 * tsl : 4 * tsl + 4, dc, :],
                start=(dc == 0),
                stop=(dc == DC - 1),
            )
        if is_q and tsl == 0:
            nc.vector.tensor_copy(qts0[eb][:], pp[:])
        elif is_q:
            qst = qstg.tile([P, TS], f32r, tag="qstage")
            nc.vector.tensor_copy(qst[:], pp[:])
            nc.gpsimd.dma_start(qdram[eb, :, ts(tsl, TS)], qst[:])
        elif tsl % 2 == 0:
            nc.vector.tensor_copy(KT[:, eb, ts(tsl, TS)], pp[:])
        else:
            nc.scalar.copy(KT[:, eb, ts(tsl, TS)], pp[:])

    # eb=0 of Q interleaves with the four transpose bursts; its weight
    # slices are loaded after the first x tiles so they don't head-block
    # the DMA rings.
    wk_view0 = wk.bitcast(f32r).rearrange("(dc dp) e -> dp dc e", dp=P)
    wr_q01 = []
    wr_k0 = None
    for tsl in range(NTS):
        emit_transposes(tsl)
        if tsl == 0:
            for eb in range(2):
                wr = wqk_pool.tile(
                    [P, DC, P], f32r, tag="wqk", name=f"wrq_{eb}"
                )
                eng = nc.sync if eb == 0 else nc.gpsimd
                eng.dma_start(wr[:], wq_view0[:, :, ts(eb, P)])
                wr_q01.append(wr)
            wr_k0 = wqk_pool.tile([P, DC, P], f32r, tag="wqk", name="wrk_0")
            nc.gpsimd.dma_start(wr_k0[:], wk_view0[:, :, ts(0, P)])
        emit_proj(wq, True, 0, wr_q01[0], tsl)
        emit_proj(wk, False, 0, wr_k0, tsl)
    # remaining Q (eb 1..7), then all K
    for w_ap, is_q in ((wq, True), (wk, False)):
        w_view = w_ap.bitcast(f32r).rearrange("(dc dp) e -> dp dc e", dp=P)
        for eb in range(EC):
            if eb == 0:
                continue
            if is_q and eb == 1:
                wr = wr_q01[1]
            else:
                wr = wqk_pool.tile([P, DC, P], f32r, tag="wqk")
                weng = (nc.sync, nc.gpsimd)[eb % 2]
                weng.dma_start(wr[:], w_view[:, :, ts(eb, P)])
            for tsl in range(NTS):
                emit_proj(w_ap, is_q, eb, wr, tsl)

    # ========== phase 1b: V = x @ Wv, evicted in place over x^T ==========
    # tb-outer with both Wv halves resident: both psums must be computed
    # before the in-place evicts may overwrite this t-block's x^T slab.
    wv_view = wv.bitcast(f32r).rearrange("(dc dp) e -> dp dc e", dp=P)
    wvhs = []
    for eh in range(E // TS):
        wvh = wvh_pool.tile([P, DC, TS], f32r, tag="wvh", name=f"wvh_{eh}")
        nc.sync.dma_start(wvh[:], wv_view[:, :, ts(eh, TS)])
        wvhs.append(wvh)
    for tb in range(TB):
        pps = []
        for eh in range(E // TS):
            pp = ps_proj.tile([P, TS], f32)
            for dc in range(DC):
                nc.tensor.matmul(
                    pp[:],
                    xv[:, tb, dc, :],
                    wvhs[eh][:, dc, :],
                    start=(dc == 0),
                    stop=(dc == DC - 1),
                )
            pps.append(pp)
        # in-place evicts over the x^T slab of this t-block (WAR: both
        # psum groups above have read the slab before these run)
        nc.scalar.copy(Vres[:, tb, ts(0, TS)], pps[0][:])
        nc.vector.tensor_copy(Vres[:, tb, ts(1, TS)], pps[1][:])

    wvh_pool.release()
    wqk_pool.release()
    xa_pool.release()
    qstg.release()
    ps_proj.release()
    ps_tp.release()

    # ================= phase 2: attention =================
    ps_s = tc.alloc_tile_pool(name="ps_s", bufs=4, space="PSUM")
    ps_o = tc.alloc_tile_pool(name="ps_o", bufs=2, space="PSUM")
    ps_sum = tc.alloc_tile_pool(name="ps_sum", bufs=2, space="PSUM")

    tc.swap_default_side()
    pt_pool = ctx.enter_context(tc.tile_pool(name="pt", bufs=5))
    rs_pool = ctx.enter_context(tc.tile_pool(name="rs", bufs=8))
    ostg = ctx.enter_context(tc.tile_pool(name="ostg", bufs=3))
    tc.swap_default_side()

    nstores = 0

    for sup in range(NTS):
        nkb = JB * sup + JB  # key blocks 0..nkb-1
        if sup == 0:
            qts = qts0
        else:
            qts = []
            for ec in range(EC):
                q1 = qt0_pool.tile([P, TS], f32r, tag="qt0", name=f"qt_{sup}_{ec}")
                nc.sync.dma_start(q1[:], qdram[ec, :, ts(sup, TS)])
                qts.append(q1)
        pt_parts = [
            pt_pool.tile([P, QB, TS], f32r, tag="pt", name=f"ptp_{sup}_0")
        ]

        # --- S^T blocks + exp + causal mask (band blocks narrowed) ---
        for k in range(nkb):
            j = k - JB * sup  # >= 0 on the diagonal band
            lo = max(j, 0) * P  # first unmasked query col in supertile
            ssp = ps_s.tile([P, TS], f32)
            for ec in range(EC):
                nc.tensor.matmul(
                    ssp[:, lo:TS],
                    KT[:, ec, ts(k, P)],
                    qts[ec][:, lo:TS],
                    start=(ec == 0),
                    stop=(ec == EC - 1),
                )
            if k // QB >= len(pt_parts):
                pt_parts.append(
                    pt_pool.tile(
                        [P, QB, TS], f32r, tag="pt",
                        name=f"ptp_{sup}_{k // QB}",
                    )
                )
            pk = pt_parts[k // QB][:, k % QB, :]
            nc.scalar.activation(pk[:, lo:TS], ssp[:, lo:TS], AF.Exp, scale=SCALE)
            if j >= 0:
                nc.vector.tensor_mul(
                    pk[:, lo : lo + P], pk[:, lo : lo + P], mask_diag[:]
                )

        # --- P @ V (+ row sums interleaved in eh=0), normalize, store ---
        rss = {}
        for eh in range(E // TS):
            for jq in range(JB):
                qb = JB * sup + jq
                nk = qb + 1
                po = ps_o.tile([P, TS], f32)
                if eh == 0:
                    pos = ps_sum.tile([P, 2], f32)
                for k in range(nk):
                    lhsT = pt_parts[k // QB][:, k % QB, ts(jq, P)]
                    nc.tensor.matmul(
                        po[:],
                        lhsT,
                        Vres[:, k, ts(eh, TS)],
                        start=(k == 0),
                        stop=(k == nk - 1),
                    )
                    if eh == 0:
                        nc.tensor.matmul(
                            pos[:],
                            lhsT,
                            ones_col[:],
                            start=(k == 0),
                            stop=(k == nk - 1),
                        )
                if eh == 0:
                    rs = rs_pool.tile(
                        [P, 1], f32, tag="rs", name=f"rs_{sup}_{jq}"
                    )
                    nc.vector.reciprocal(rs[:], pos[:, 0:1])
                    rss[jq] = rs
                ost = ostg.tile([P, TS], f32, tag="ostage")
                if eh == 0:
                    nc.scalar.activation(
                        ost[:], po[:], AF.Copy, scale=rss[jq][:]
                    )
                else:
                    nc.vector.tensor_scalar_mul(ost[:], po[:], rss[jq][:])
                nc.gpsimd.dma_start(out[ts(qb, P), ts(eh, TS)], ost[:])
                nstores += 1

    ps_sum.release()
    ps_o.release()
    ps_s.release()


def build_program():
    from contextlib import ExitStack

    import concourse.bacc as bacc
    import concourse.tile as tile
    from concourse import mybir

    nc = bacc.Bacc("TRN2", target_bir_lowering=False, debug=False)
    f32 = mybir.dt.float32
    x = nc.dram_tensor("x", [T, D], f32, kind="ExternalInput").ap()
    wq = nc.dram_tensor("Wq", [D, E], f32, kind="ExternalInput").ap()
    wk = nc.dram_tensor("Wk", [D, E], f32, kind="ExternalInput").ap()
    wv = nc.dram_tensor("Wv", [D, E], f32, kind="ExternalInput").ap()
    out = nc.dram_tensor("out", [T, E], f32, kind="ExternalOutput").ap()

    with tile.TileContext(nc) as tc:
        with ExitStack() as ctx:
            _attention_kernel(ctx, tc, out, x, wq, wk, wv)
    nc.compile()
    return nc


def kernel(x, Wq, Wk, Wv, _trace=False):
    from concourse.bass_utils import run_bass_kernel_spmd

    x = np.ascontiguousarray(np.asarray(x), dtype=np.float32)
    Wq = np.ascontiguousarray(np.asarray(Wq), dtype=np.float32)
    Wk = np.ascontiguousarray(np.asarray(Wk), dtype=np.float32)
    Wv = np.ascontiguousarray(np.asarray(Wv), dtype=np.float32)
    assert x.shape == (N_CORES, T, D), x.shape

    nc = build_program()
    in_maps = [
        {"x": np.ascontiguousarray(x[b]), "Wq": Wq, "Wk": Wk, "Wv": Wv}
        for b in range(N_CORES)
    ]
    last_err = None
    for attempt in range(3):
        try:
            res = run_bass_kernel_spmd(
                nc, in_maps, core_ids=list(range(N_CORES)), trace=_trace
            )
            break
        except Exception as e:  # transient device wedge: retry
            last_err = e
            import time

            time.sleep(5.0 * (attempt + 1))
    else:
        raise last_err
    out = np.stack([res.results[b]["out"] for b in range(N_CORES)], axis=0)
    if _trace:
        kernel.last_results = res
    return out


kernel.last_results = None


# revision 19
# speedup vs baseline: 1.0666x; 1.0257x over previous
"""Causal attention (QKV proj + softmax(QK^T/sqrt(d))V) on 8 TRN2 NeuronCores.

Sharding: data-parallel over batch (B=8, one batch element per core).
Per-core kernel, all matmuls in float32r (fast fp32 mode; measured
faster per column than bf16 on this part -- bf16 triggers FWL weight
loads that collide with moving-operand streaming):
  warmup: a dozen dummy N=512 matmuls at program start trip the PE HAM
          clock gate to 8/8 before real work arrives; single warm
          matmuls are sprinkled into long transpose stretches so the
          activity monitor never re-throttles.
  phase 1a: x -> x^T via PE transposes, INTERLEAVED with the eb=0 Q
          projections so the PE never sits idle on DMA; Q^T -> DRAM
          scratch (supertile 0 evicted straight to SBUF); K^T -> SBUF
          resident.
  phase 1b: V = x @ Wv evicted IN PLACE over the x^T slab of the same
          t-block (x^T morphs into resident V, no roundtrip).
  phase 2: per 512-wide query supertile: S^T = K Q^T narrowed on the
          diagonal band to the unmasked query range, exp on ACT with
          fused 1/sqrt(D) scale, 128x128 triangular mask on diagonal
          blocks only, P@V with interleaved ones-matmul row sums on PE,
          reciprocal normalize, store (rings rotated).
"""

import numpy as np

T = 2048
D = 1024
E = 1024
N_CORES = 8
P = 128
TS = 512  # t-slice / supertile width
SCALE = 1.0 / 32.0  # 1/sqrt(D)

DC = D // P  # 8 d-chunks
EC = E // P  # 8 e-chunks
TB = T // P  # 16 t-blocks of 128
NTS = T // TS  # 4 t-slices of 512
JB = TS // P  # 4 q-blocks per supertile
QB = TB // 4  # pt part size in k-blocks


def _attention_kernel(ctx, tc, out, x, wq, wk, wv):
    import concourse.bass as bass
    from concourse import mybir
    from concourse.bass import ts
    from concourse.masks import make_identity

    nc = tc.nc
    f32 = mybir.dt.float32
    f32r = mybir.dt.float32r
    AF = mybir.ActivationFunctionType

    # ---- DRAM scratch ----
    dram = ctx.enter_context(tc.tile_pool(name="dram", bufs=1, space="DRAM"))
    qdram = dram.tile([EC, P, T], f32r)  # Q^T[e,t], e = ec*128 + ep

    # ---- left-side SBUF pools ----
    const = ctx.enter_context(tc.tile_pool(name="const", bufs=1))
    ones_f32 = const.tile([P, 2], f32)
    nc.vector.memset(ones_f32[:], 1.0)
    ones_col = const.tile([P, 2], f32r)
    nc.vector.tensor_copy(ones_col[:], ones_f32[:])
    # warm the ACT exp table set at program start (off the critical path)
    exp_warm = const.tile([P, 2], f32)
    nc.scalar.activation(exp_warm[:], ones_f32[:], AF.Exp)
    identity_f32 = const.tile([P, P], f32)
    make_identity(nc, identity_f32[:])
    identity = const.tile([P, P], f32r)
    nc.vector.tensor_copy(identity[:], identity_f32[:])
    identity_bf = const.tile([P, P], mybir.dt.bfloat16)
    nc.vector.tensor_copy(identity_bf[:], identity_f32[:])

    # 128x128 triangular mask (keep where col - p >= 0) for the diagonal
    # 128-col sub-block of each narrowed band S^T block.
    mask_f32 = const.tile([P, P], f32)
    nc.gpsimd.memset(mask_f32[:], 1.0)
    nc.gpsimd.affine_select(
        out=mask_f32[:],
        in_=mask_f32[:],
        compare_op=mybir.AluOpType.is_ge,
        fill=0.0,
        base=0,
        pattern=[[1, P]],
        channel_multiplier=-1,
    )
    mask_diag = const.tile([P, P], f32r)
    nc.vector.tensor_copy(mask_diag[:], mask_f32[:])

    # PE warmup fodder (memset garbage, result never read)
    wu_f32 = const.tile([P, 256], f32)
    nc.gpsimd.memset(wu_f32[:], 0.0)
    wu_rhs = wu_f32[:].bitcast(f32r)
    wu_sink = const.tile([P, 2], f32)

    kt_pool = ctx.enter_context(tc.tile_pool(name="ktres", bufs=1))
    KT = kt_pool.tile([P, EC, T], f32r)  # K^T[e, t], e = ec*128 + ep

    # supertile-0 Q tiles prefetch here (left side, not gated by reuse)
    qt0_pool = ctx.enter_context(tc.tile_pool(name="qt0", bufs=10))

    # ---- right-side work pools ----
    tc.swap_default_side()
    xv_pool = ctx.enter_context(tc.tile_pool(name="xv", bufs=1))
    # x^T t-block-major; after phase 1 each slab is overwritten in place
    # with V[tb] so this same tile is the resident V in phase 2.
    xv = xv_pool.tile([P, TB, DC, P], f32r)  # [dp, tb, dc, tl]
    Vres = xv[:].rearrange("p tb dc e -> p tb (dc e)")  # V[t, e] view
    qstg = tc.alloc_tile_pool(name="qstg", bufs=2)
    xa_pool = tc.alloc_tile_pool(name="xa", bufs=8)
    wqk_pool = tc.alloc_tile_pool(name="wqk", bufs=3)
    wvh_pool = tc.alloc_tile_pool(name="wvh", bufs=2)
    tc.swap_default_side()

    # ---- PSUM pools for phases 0/1 ----
    ps_tp = tc.alloc_tile_pool(name="ps_tp", bufs=3, space="PSUM")
    ps_proj = tc.alloc_tile_pool(name="ps_proj", bufs=5, space="PSUM")

    def warm_mms(n, sink=False):
        """n dummy N=256 matmuls to keep the PE HAM activity window busy."""
        for g in range(n):
            pp = ps_proj.tile([P, TS], f32, tag="pp", name=f"wu{warm_mms.idx}")
            warm_mms.idx += 1
            nc.tensor.matmul(
                pp[:, 0:256], identity[:], wu_rhs, start=True, stop=True
            )
            if sink and g == n - 1:
                nc.vector.tensor_copy(wu_sink[:], pp[:, 0:2])

    warm_mms.idx = 0

    # ===== PE warmup: ~5us of dummy matmuls trips HAM to 8/8 =====
    warm_mms(16, sink=True)

    wq_view0 = wq.bitcast(f32r).rearrange("(dc dp) e -> dp dc e", dp=P)

    # ===== phase 0/1a fused =====
    # x^T transposes are interleaved with HALF-WIDTH (N=256) Q/K eb=0
    # projection chains after every 2 t-blocks: real matmul work every
    # ~2us keeps the HAM clock gate at 8/8 through the load phase
    # (PE-transposes do not count as activity), at ~zero overhead
    # (N=256 f32r still streams 1 col/cycle).
    xvflat = xv[:].rearrange("p tb dc e -> p tb (dc e)")

    def emit_transpose_tb(tb):
        pieces = []
        for g in range(4):
            gs = slice(g * 256, (g + 1) * 256)
            xa = xa_pool.tile([P, 256], f32r, tag="xa", name=f"xa_{tb}_{g}")
            eng = (nc.sync, nc.gpsimd)[(4 * tb + g) % 2]
            eng.dma_start(xa[:], x[ts(tb, P), gs].bitcast(f32r))
            pieces.append(xa)
        for h in range(2):
            ptw = ps_tp.tile([P, TS], f32r, tag="ptw", name=f"ptw_{tb}_{h}")
            for dd in range(4):
                nc.tensor.transpose(
                    ptw[:, ts(dd, P)],
                    pieces[2 * h + dd // 2][:, ts(dd % 2, P)],
                    identity[:],
                )
            if h == 0:
                nc.vector.tensor_copy(xvflat[:, tb, 0:TS], ptw[:])
            else:
                nc.scalar.copy(xvflat[:, tb, TS : 2 * TS], ptw[:])

    qts0 = [
        qt0_pool.tile([P, TS], f32r, tag="qt0", name=f"qts0_{ec}")
        for ec in range(EC)
    ]

    def emit_proj(w_ap, is_q, eb, wr, tsl):
        pp = ps_proj.tile([P, TS], f32)
        for dc in range(DC):
            nc.tensor.matmul(
                pp[:],
                wr[:, dc, :],
                xv[:, 4 * tsl : 4 * tsl + 4, dc, :],
                start=(dc == 0),
                stop=(dc == DC - 1),
            )
        evict_proj(pp, is_q, eb, tsl)

    def evict_proj(pp, is_q, eb, tsl):
        if is_q and tsl == 0:
            nc.vector.tensor_copy(qts0[eb][:], pp[:])
        elif is_q:
            qst = qstg.tile([P, TS], f32r, tag="qstage")
            nc.vector.tensor_copy(qst[:], pp[:])
            nc.gpsimd.dma_start(qdram[eb, :, ts(tsl, TS)], qst[:])
        elif tsl % 2 == 0:
            nc.vector.tensor_copy(KT[:, eb, ts(tsl, TS)], pp[:])
        else:
            nc.scalar.copy(KT[:, eb, ts(tsl, TS)], pp[:])

    # eb=0 weight slices for Q and K, loaded first
    wk_view0 = wk.bitcast(f32r).rearrange("(dc dp) e -> dp dc e", dp=P)
    wr_q0 = wqk_pool.tile([P, DC, P], f32r, tag="wqk", name="wrq_0")
    nc.gpsimd.dma_start(wr_q0[:], wq_view0[:, :, ts(0, P)])
    wr_k0 = wqk_pool.tile([P, DC, P], f32r, tag="wqk", name="wrk_0")
    nc.sync.dma_start(wr_k0[:], wk_view0[:, :, ts(0, P)])
    wr_q1 = wqk_pool.tile([P, DC, P], f32r, tag="wqk", name="wrq_1")
    nc.gpsimd.dma_start(wr_q1[:], wq_view0[:, :, ts(1, P)])
    wr_q01 = [wr_q0, wr_q1]

    for tsl in range(NTS):
        ppq = ps_proj.tile([P, TS], f32, tag="pp", name=f"ppq0_{tsl}")
        ppk = ps_proj.tile([P, TS], f32, tag="pp", name=f"ppk0_{tsl}")
        for h2 in range(2):
            emit_transpose_tb(4 * tsl + 2 * h2)
            emit_transpose_tb(4 * tsl + 2 * h2 + 1)
            for pp, wr in ((ppq, wr_q0), (ppk, wr_k0)):
                for dc in range(DC):
                    nc.tensor.matmul(
                        pp[:, ts(h2, 256)],
                        wr[:, dc, :],
                        xv[:, 4 * tsl + 2 * h2 : 4 * tsl + 2 * h2 + 2, dc, :],
                        start=(dc == 0),
                        stop=(dc == DC - 1),
                    )
        evict_proj(ppq, True, 0, tsl)
        evict_proj(ppk, False, 0, tsl)

    # remaining Q (eb 1..7), then all K
    for w_ap, is_q in ((wq, True), (wk, False)):
        w_view = w_ap.bitcast(f32r).rearrange("(dc dp) e -> dp dc e", dp=P)
        for eb in range(EC):
            if eb == 0:
                continue
            if is_q and eb == 1:
                wr = wr_q01[1]
            else:
                wr = wqk_pool.tile([P, DC, P], f32r, tag="wqk")
                weng = (nc.sync, nc.gpsimd)[eb % 2]
                weng.dma_start(wr[:], w_view[:, :, ts(eb, P)])
            for tsl in range(NTS):
                emit_proj(w_ap, is_q, eb, wr, tsl)

    # ========== phase 1b: V = x @ Wv, evicted in place over x^T ==========
    # tb-outer with both Wv halves resident: both psums must be computed
    # before the in-place evicts may overwrite this t-block's x^T slab.
    wv_view = wv.bitcast(f32r).rearrange("(dc dp) e -> dp dc e", dp=P)
    wvhs = []
    for eh in range(E // TS):
        wvh = wvh_pool.tile([P, DC, TS], f32r, tag="wvh", name=f"wvh_{eh}")
        nc.sync.dma_start(wvh[:], wv_view[:, :, ts(eh, TS)])
        wvhs.append(wvh)
    for tb in range(TB):
        pps = []
        for eh in range(E // TS):
            pp = ps_proj.tile([P, TS], f32)
            for dc in range(DC):
                nc.tensor.matmul(
                    pp[:],
                    xv[:, tb, dc, :],
                    wvhs[eh][:, dc, :],
                    start=(dc == 0),
                    stop=(dc == DC - 1),
                )
            pps.append(pp)
        # in-place evicts over the x^T slab of this t-block (WAR: both
        # psum groups above have read the slab before these run)
        nc.scalar.copy(Vres[:, tb, ts(0, TS)], pps[0][:])
        nc.vector.tensor_copy(Vres[:, tb, ts(1, TS)], pps[1][:])

    wvh_pool.release()
    wqk_pool.release()
    xa_pool.release()
    qstg.release()
    ps_proj.release()
    ps_tp.release()

    # ================= phase 2: attention =================
    ps_s = tc.alloc_tile_pool(name="ps_s", bufs=4, space="PSUM")
    ps_o = tc.alloc_tile_pool(name="ps_o", bufs=2, space="PSUM")
    ps_sum = tc.alloc_tile_pool(name="ps_sum", bufs=2, space="PSUM")

    tc.swap_default_side()
    pt_pool = ctx.enter_context(tc.tile_pool(name="pt", bufs=5))
    rs_pool = ctx.enter_context(tc.tile_pool(name="rs", bufs=8))
    ostg = ctx.enter_context(tc.tile_pool(name="ostg", bufs=3))
    tc.swap_default_side()

    nstores = 0

    for sup in range(NTS):
        nkb = JB * sup + JB  # key blocks 0..nkb-1
        if sup == 0:
            qts = qts0
        else:
            qts = []
            for ec in range(EC):
                q1 = qt0_pool.tile([P, TS], f32r, tag="qt0", name=f"qt_{sup}_{ec}")
                nc.sync.dma_start(q1[:], qdram[ec, :, ts(sup, TS)])
                qts.append(q1)
        pt_parts = [
            pt_pool.tile([P, QB, TS], f32r, tag="pt", name=f"ptp_{sup}_0")
        ]

        # --- S^T blocks + exp + causal mask (band blocks narrowed) ---
        for k in range(nkb):
            j = k - JB * sup  # >= 0 on the diagonal band
            lo = max(j, 0) * P  # first unmasked query col in supertile
            ssp = ps_s.tile([P, TS], f32)
            for ec in range(EC):
                nc.tensor.matmul(
                    ssp[:, lo:TS],
                    KT[:, ec, ts(k, P)],
                    qts[ec][:, lo:TS],
                    start=(ec == 0),
                    stop=(ec == EC - 1),
                )
            if k // QB >= len(pt_parts):
                pt_parts.append(
                    pt_pool.tile(
                        [P, QB, TS], f32r, tag="pt",
                        name=f"ptp_{sup}_{k // QB}",
                    )
                )
            pk = pt_parts[k // QB][:, k % QB, :]
            nc.scalar.activation(pk[:, lo:TS], ssp[:, lo:TS], AF.Exp, scale=SCALE)
            if j >= 0:
                nc.vector.tensor_mul(
                    pk[:, lo : lo + P], pk[:, lo : lo + P], mask_diag[:]
                )

        # --- P @ V (+ row sums interleaved in eh=0), normalize, store ---
        rss = {}
        for eh in range(E // TS):
            for jq in range(JB):
                qb = JB * sup + jq
                nk = qb + 1
                po = ps_o.tile([P, TS], f32)
                if eh == 0:
                    pos = ps_sum.tile([P, 2], f32)
                for k in range(nk):
                    lhsT = pt_parts[k // QB][:, k % QB, ts(jq, P)]
                    nc.tensor.matmul(
                        po[:],
                        lhsT,
                        Vres[:, k, ts(eh, TS)],
                        start=(k == 0),
                        stop=(k == nk - 1),
                    )
                    if eh == 0:
                        nc.tensor.matmul(
                            pos[:],
                            lhsT,
                            ones_col[:],
                            start=(k == 0),
                            stop=(k == nk - 1),
                        )
                if eh == 0:
                    rs = rs_pool.tile(
                        [P, 1], f32, tag="rs", name=f"rs_{sup}_{jq}"
                    )
                    nc.vector.reciprocal(rs[:], pos[:, 0:1])
                    rss[jq] = rs
                ost = ostg.tile([P, TS], f32, tag="ostage")
                if eh == 0:
                    nc.scalar.activation(
                        ost[:], po[:], AF.Copy, scale=rss[jq][:]
                    )
                else:
                    nc.vector.tensor_scalar_mul(ost[:], po[:], rss[jq][:])
                nc.gpsimd.dma_start(out[ts(qb, P), ts(eh, TS)], ost[:])
                nstores += 1

    ps_sum.release()
    ps_o.release()
    ps_s.release()


def build_program():
    from contextlib import ExitStack

    import concourse.bacc as bacc
    import concourse.tile as tile
    from concourse import mybir

    nc = bacc.Bacc("TRN2", target_bir_lowering=False, debug=False)
    f32 = mybir.dt.float32
    x = nc.dram_tensor("x", [T, D], f32, kind="ExternalInput").ap()
    wq = nc.dram_tensor("Wq", [D, E], f32, kind="ExternalInput").ap()
    wk = nc.dram_tensor("Wk", [D, E], f32, kind="ExternalInput").ap()
    wv = nc.dram_tensor("Wv", [D, E], f32, kind="ExternalInput").ap()
    out = nc.dram_tensor("out", [T, E], f32, kind="ExternalOutput").ap()

    with tile.TileContext(nc) as tc:
        with ExitStack() as ctx:
            _attention_kernel(ctx, tc, out, x, wq, wk, wv)
    nc.compile()
    return nc


def kernel(x, Wq, Wk, Wv, _trace=False):
    from concourse.bass_utils import run_bass_kernel_spmd

    x = np.ascontiguousarray(np.asarray(x), dtype=np.float32)
    Wq = np.ascontiguousarray(np.asarray(Wq), dtype=np.float32)
    Wk = np.ascontiguousarray(np.asarray(Wk), dtype=np.float32)
    Wv = np.ascontiguousarray(np.asarray(Wv), dtype=np.float32)
    assert x.shape == (N_CORES, T, D), x.shape

    nc = build_program()
    in_maps = [
        {"x": np.ascontiguousarray(x[b]), "Wq": Wq, "Wk": Wk, "Wv": Wv}
        for b in range(N_CORES)
    ]
    last_err = None
    for attempt in range(3):
        try:
            res = run_bass_kernel_spmd(
                nc, in_maps, core_ids=list(range(N_CORES)), trace=_trace
            )
            break
        except Exception as e:  # transient device wedge: retry
            last_err = e
            import time

            time.sleep(5.0 * (attempt + 1))
    else:
        raise last_err
    out = np.stack([res.results[b]["out"] for b in range(N_CORES)], axis=0)
    if _trace:
        kernel.last_results = res
    return out


kernel.last_results = None
